# revision 6
# baseline (speedup 1.0000x reference)
"""Trainium2 Bass kernel for causal multi-head attention (AbstractNaiveMHA).

Problem shapes (hardcoded per the harness contract):
  x: [B=2, S=2048, D=1024] f32, mask: [B, S] int32 (all ones)
  Wq/Wk/Wv: [H=16, 64, 1024], bq/bk/bv: [16, 64], Wo: [1024, 1024], bo: [1024]

Sharding over 8 NeuronCores: core c -> batch b = c // 4, head group
g = c % 4 (heads 4g..4g+3).  Each core computes its 4 heads' attention
and a partial output projection through its column slice of Wo; the
host sums the 4 partials per batch and adds bo (the "all-reduce").

v2 design notes (all matmuls bf16, rel tol 2e-2 permits it):
  - t-chunked pipeline: for each 512-token i-chunk t: project q/k/v for
    chunk t, run attention for chunk t against j-tiles 0..4t+3, then the
    output projection for chunk t.  This overlaps the ACT-engine exp
    (the phase-2 bottleneck) with projection/output matmuls and spreads
    the yT DMA across the run.
  - Scores are computed transposed (S^T[j,i]) with the two heads of a
    pair in PE row-quadrants 0:63/64:127; quadrant matmuls dual-issue
    (~95ns instead of 213ns for f=512), so k=64 costs no extra.
  - V path: vT computed v-major (weights stationary), then
    dma_start_transpose (idle DMA engines) produces the token-major
    [V_even |ones| V_odd] tiles that the AV matmul needs.
  - Causal handling: diagonal j-tiles restrict scores+exp to i >= 128r
    (saves PE stream and ACT elems); a single [zeros|tri] mask multiply
    zeroes the stale region of the es tile (ring bufs pre-zeroed once so
    stale data is always finite).
  - Softmax denominator rides free in the AV matmul via ones columns:
    av[0]=[out_e|den_e], av[1]=[den_o|out_o].  Normalization uses
    cross-partition reciprocal_approx_fast (InstReciprocal is ~7ns/elem,
    5x slower) plus two aligned muls - no flip copies.
"""

import os

import numpy as np
import ml_dtypes

import concourse.bass as bass
import concourse.mybir as mybir
import concourse.tile as tile
from concourse import bacc
from concourse.bass import ts, ds
from concourse.bass_utils import run_bass_kernel_spmd

B, S, D = 2, 2048, 1024
H, KD = 16, 64
P = 128
NT = S // 512      # 4 i-chunks of 512
NJ = S // P        # 16 j-tiles of 128
DCH = D // P       # 8 contraction chunks for the projections
N_CORES = 8
HEADS_PER_CORE = 4

F32 = mybir.dt.float32
BF16 = mybir.dt.bfloat16
EXP_SCALE = 1.0 / float(np.sqrt(np.float32(KD)))


def build_module(repeat: int = 1):
    """Build the single-core Bass module (same program on all 8 cores)."""
    nc = bacc.Bacc("TRN2", target_bir_lowering=False, debug=False)

    xT = nc.dram_tensor("xT", [D, S], BF16, kind="ExternalInput").ap()
    wq = nc.dram_tensor("wq", [2, D, P], BF16, kind="ExternalInput").ap()
    wk = nc.dram_tensor("wk", [2, D, P], BF16, kind="ExternalInput").ap()
    wv = nc.dram_tensor("wv", [D, 2 * P], BF16, kind="ExternalInput").ap()
    bqk = nc.dram_tensor("bqk", [2, 2, P], F32, kind="ExternalInput").ap()
    bv = nc.dram_tensor("bv", [P, 2], F32, kind="ExternalInput").ap()
    wo = nc.dram_tensor("wo", [P, 2, D], BF16, kind="ExternalInput").ap()
    zmask = nc.dram_tensor("zmask", [P, 1, 512], BF16,
                           kind="ExternalInput").ap()
    yT = nc.dram_tensor("yT", [D, S], F32, kind="ExternalOutput").ap()

    Exp = mybir.ActivationFunctionType.Exp

    with tile.TileContext(nc) as tc:
        with (
            tc.tile_pool(name="cache", bufs=1) as cache,
            tc.tile_pool(name="e_pool", bufs=4) as e_pool,
            tc.tile_pool(name="c_pool", bufs=4) as c_pool,
            tc.tile_pool(name="r_pool", bufs=2) as r_pool,
            tc.tile_pool(name="y_pool", bufs=3) as y_pool,
            tc.tile_pool(name="sp_ps", bufs=2, space="PSUM") as sp_ps,
            tc.tile_pool(name="av_ps", bufs=2, space="PSUM") as av_ps,
            tc.tile_pool(name="pjq_ps", bufs=1, space="PSUM") as pjq_ps,
            tc.tile_pool(name="pjy_ps", bufs=1, space="PSUM") as pjy_ps,
        ):
            def body():
                # ---- persistent caches ----
                xT_sb = cache.tile([P, DCH, S], BF16, tag="xT")
                nc.sync.dma_start(xT_sb[:], xT.rearrange("(c p) s -> p c s", p=P))
                wq_sb = cache.tile([P, 2, DCH, P], BF16, tag="wq")
                nc.sync.dma_start(wq_sb[:], wq.rearrange("r (c p) m -> p r c m", p=P))
                wk_sb = cache.tile([P, 2, DCH, P], BF16, tag="wk")
                nc.sync.dma_start(wk_sb[:], wk.rearrange("r (c p) m -> p r c m", p=P))
                wv_sb = cache.tile([P, DCH, 2 * P], BF16, tag="wv")
                nc.sync.dma_start(wv_sb[:], wv.rearrange("(c p) n -> p c n", p=P))
                bqk_sb = cache.tile([P, 2, 2], F32, tag="bqk")
                nc.sync.dma_start(bqk_sb[:], bqk.rearrange("q r p -> p q r"))
                bv_sb = cache.tile([P, 2], F32, tag="bv")
                nc.sync.dma_start(bv_sb[:], bv[:])
                wo_sb = cache.tile([P, 2, D], BF16, tag="wo")
                nc.sync.dma_start(wo_sb[:], wo[:])
                z_sb = cache.tile([P, 1, 512], BF16, tag="z")
                nc.sync.dma_start(z_sb[:], zmask[:])

                qT_sb = cache.tile([P, 2, S], BF16, tag="qT")
                kT_sb = cache.tile([P, 2, S], BF16, tag="kT")
                vT_sb = cache.tile([P, 2, S], BF16, tag="vT")
                # per (j-tile, pair): [V_even | ones | V_odd] x 64 cols
                vones = cache.tile([P, NJ, 2, 192], BF16, tag="vones")
                nc.vector.memset(vones[:, :, :, 64:128], 1.0)

                # pre-zero the es ring so stale regions are always finite
                for i in range(4):
                    e0 = e_pool.tile([P, 2, 512], BF16, tag="e",
                                     name=f"einit{i}")
                    nc.vector.memset(e0[:], 0.0)

                # ---- work-unit emitters ----
                def qk_group(t, pr, w_sb, dstT, bcol):
                    ps = pjq_ps.tile([P, 512], F32, tag="pj",
                                     name=f"qk{t}{pr}{bcol}")
                    for c in range(DCH):
                        nc.tensor.matmul(
                            ps[:], w_sb[:, pr, c, :],
                            xT_sb[:, c, ts(t, 512)],
                            start=(c == 0), stop=(c == DCH - 1))
                    nc.vector.tensor_scalar_add(
                        dstT[:, pr, ts(t, 512)], ps[:],
                        bqk_sb[:, pr, bcol:bcol + 1])

                def v_group(t, vt):
                    ps = pjq_ps.tile([P, 512], F32, tag="pj",
                                     name=f"v{t}{vt}")
                    for c in range(DCH):
                        nc.tensor.matmul(
                            ps[:], wv_sb[:, c, ds(P * vt, P)],
                            xT_sb[:, c, ts(t, 512)],
                            start=(c == 0), stop=(c == DCH - 1))
                    nc.vector.tensor_scalar_add(
                        vT_sb[:, vt, ts(t, 512)], ps[:],
                        bv_sb[:, vt:vt + 1])
                    # transposes for this vtile's head pair (pr == vt) only:
                    # the other vtile's chunk-t data isn't drained yet
                    for r in range(4):
                        jt = 4 * t + r
                        nc.sync.dma_start_transpose(
                            vones[:, jt, vt, 0:64],
                            vT_sb[0:64, vt, ts(jt, P)])
                        nc.sync.dma_start_transpose(
                            vones[:, jt, vt, 128:192],
                            vT_sb[64:P, vt, ts(jt, P)])

                def proj_fillers(t):
                    fs = []
                    for pr in range(2):
                        for (w_sb, dstT, bcol) in ((wq_sb, qT_sb, 0),
                                                   (wk_sb, kT_sb, 1)):
                            fs.append(lambda t=t, pr=pr, w_sb=w_sb,
                                      dstT=dstT, bcol=bcol:
                                      qk_group(t, pr, w_sb, dstT, bcol))
                    for vt in range(2):
                        fs.append(lambda t=t, vt=vt: v_group(t, vt))
                    return fs

                def y_group(t, dt_, cc):
                    yps = pjy_ps.tile([P, 512], F32, tag="py",
                                      name=f"y{t}{dt_}")
                    nc.tensor.matmul(yps[:], wo_sb[:, 0, ts(dt_, P)],
                                     cc[0][:], start=True, stop=False)
                    nc.tensor.matmul(yps[:], wo_sb[:, 1, ts(dt_, P)],
                                     cc[1][:], start=False, stop=True)
                    y_sb = y_pool.tile([P, 512], F32, tag="y",
                                       name=f"ysb{t}{dt_}")
                    nc.vector.tensor_copy(y_sb[:], yps[:])
                    nc.sync.dma_start(yT[ts(dt_, P), ts(t, 512)], y_sb[:])

                def phase3_fillers(t, cc):
                    return [lambda t=t, dt_=dt_, cc=cc: y_group(t, dt_, cc)
                            for dt_ in range(DCH)]

                def attn_visit(t, pr, jt, njt, av):
                    r = jt - 4 * t
                    f0 = P * r if r > 0 else 0
                    sp = sp_ps.tile([P, 2, 512], F32, tag="sp",
                                    name=f"sp{t}{pr}{jt}")
                    for hi in range(2):
                        nc.tensor.matmul(
                            sp[:, hi, f0:512],
                            kT_sb[ds(64 * hi, 64), pr, ts(jt, P)],
                            qT_sb[ds(64 * hi, 64), pr,
                                  ds(512 * t + f0, 512 - f0)],
                            start=True, stop=True)
                    es = e_pool.tile([P, 2, 512], BF16, tag="e",
                                     name=f"es{t}{pr}{jt}")
                    nc.scalar.activation(
                        es[:, :, f0:512], sp[:, :, f0:512],
                        Exp, scale=EXP_SCALE)
                    if r >= 0:
                        w_ = P * (r + 1)
                        nc.vector.tensor_mul(
                            es[:, :, 0:w_], es[:, :, 0:w_],
                            z_sb[:, :, 512 - w_:512].to_broadcast(
                                (P, 2, w_)))
                    for hi in range(2):
                        # lhsT [V_even | ones] or [ones | V_odd]
                        nc.tensor.matmul(
                            av[hi][:],
                            vones[:, jt, pr, ds(64 * hi, P)],
                            es[:, hi, :],
                            start=(jt == 0), stop=(jt == njt - 1))

                def normalize(t, pr, av):
                    # av[0]=[out_e|den_e], av[1]=[den_o|out_o].
                    # cross-partition moves must be tensor_copy (DVE
                    # compute ops are lane-locked); 2 crossings is the
                    # minimum since out_h/den_h sit on opposite halves.
                    w = r_pool.tile([P, 512], F32, tag="w", name=f"w{t}{pr}")
                    nc.vector.tensor_copy(w[0:64, :], av[0][64:P, :])
                    nc.vector.tensor_copy(w[64:P, :], av[1][0:64, :])
                    rc = r_pool.tile([P, 512], F32, tag="rc",
                                     name=f"rc{t}{pr}")
                    nc.vector.reciprocal_approx_fast(rc[:], w[:])
                    concat = c_pool.tile([P, 512], BF16, tag="cc",
                                         name=f"cc{t}{pr}")
                    nc.vector.tensor_mul(
                        concat[0:64, :], av[0][0:64, :], rc[0:64, :])
                    nc.vector.tensor_mul(
                        concat[64:P, :], av[1][64:P, :], rc[64:P, :])
                    return concat

                # chunk-0 projections up front, then per chunk t: attention
                # visits with proj(t+1) and output-proj(t-1) groups
                # interleaved so the PE has filler work while ACT runs exp.
                for f in proj_fillers(0):
                    f()
                concats = {}
                for t in range(NT):
                    fillers = []
                    if t + 1 < NT:
                        fillers += proj_fillers(t + 1)
                    if t - 1 >= 0:
                        fillers += phase3_fillers(t - 1, concats[t - 1])
                    njt = 4 * t + 4
                    visits = [(pr, jt) for pr in range(2)
                              for jt in range(njt)]
                    nf = len(fillers)
                    nv = len(visits)
                    emitted = 0
                    av = None
                    for vi, (pr, jt) in enumerate(visits):
                        if jt == 0:
                            av = [av_ps.tile([P, 512], F32, tag="av",
                                             name=f"av{t}{pr}{hi}")
                                  for hi in range(2)]
                        attn_visit(t, pr, jt, njt, av)
                        if jt == njt - 1:
                            concats.setdefault(t, {})[pr] = \
                                normalize(t, pr, av)
                        while emitted * nv < (vi + 1) * nf:
                            fillers[emitted]()
                            emitted += 1
                for f in phase3_fillers(NT - 1, concats[NT - 1]):
                    f()

            if repeat > 1:
                with tc.For_i(0, repeat, 1):
                    body()
            else:
                body()

    nc.compile()
    return nc


def make_in_maps(inputs):
    bf16 = ml_dtypes.bfloat16
    x = np.asarray(inputs["x"], dtype=np.float32)
    Wq = np.asarray(inputs["Wq"], dtype=np.float32)
    bq = np.asarray(inputs["bq"], dtype=np.float32)
    Wk = np.asarray(inputs["Wk"], dtype=np.float32)
    bk = np.asarray(inputs["bk"], dtype=np.float32)
    Wv = np.asarray(inputs["Wv"], dtype=np.float32)
    bv = np.asarray(inputs["bv"], dtype=np.float32)
    Wo = np.asarray(inputs["Wo"], dtype=np.float32)

    # [zeros(384) | tri(128)]: col slice [512-128(r+1):] gives r zero
    # blocks followed by the triangular mask for the diagonal j-tile
    jj = np.arange(P)[:, None]
    ii = np.arange(P)[None, :]
    z = np.zeros((P, 1, 512), dtype=np.float32)
    z[:, 0, 384:512] = (jj <= ii).astype(np.float32)
    z = z.astype(bf16)

    in_maps = []
    for c in range(N_CORES):
        b = c // 4
        g = c % 4
        heads = list(range(HEADS_PER_CORE * g, HEADS_PER_CORE * (g + 1)))
        xT = np.ascontiguousarray(x[b].T).astype(bf16)           # [D, S]
        wq_c = np.stack([
            np.ascontiguousarray(Wq[heads[2 * p:2 * p + 2]].reshape(P, D).T)
            for p in range(2)]).astype(bf16)                      # [2, D, 128]
        wk_c = np.stack([
            np.ascontiguousarray(Wk[heads[2 * p:2 * p + 2]].reshape(P, D).T)
            for p in range(2)]).astype(bf16)
        wv_c = np.ascontiguousarray(
            Wv[heads].reshape(2 * P, D).T).astype(bf16)           # [D, 256]
        bqk = np.stack([
            bq[heads].reshape(2, P),
            bk[heads].reshape(2, P)])                             # [qk, pr, P]
        bv_c = np.stack([bv[heads[0:2]].reshape(P),
                         bv[heads[2:4]].reshape(P)], axis=1)      # [P, 2]
        # wo[c, p, d] = Wo[d, 256 g + 128 p + c]
        wo_g = Wo[:, 2 * P * g:2 * P * (g + 1)]                   # [D, 256]
        wo_c = np.ascontiguousarray(
            wo_g.T.reshape(2, P, D).transpose(1, 0, 2)).astype(bf16)
        in_maps.append({
            "xT": xT, "wq": wq_c, "wk": wk_c, "wv": wv_c,
            "bqk": np.ascontiguousarray(bqk.transpose(1, 0, 2)),  # [pr, qk, P]
            "bv": np.ascontiguousarray(bv_c), "wo": wo_c, "zmask": z,
        })
    return in_maps


_cached = {}


def _get_module(repeat: int = 1):
    if repeat not in _cached:
        _cached[repeat] = build_module(repeat)
    return _cached[repeat]


def run_cores(inputs, repeat: int = 1):
    nc = _get_module(repeat)
    in_maps = make_in_maps(inputs)
    res = run_bass_kernel_spmd(nc, in_maps, core_ids=list(range(N_CORES)))
    return res.results


def assemble(results, bo):
    y = np.zeros((B, S, D), dtype=np.float32)
    for c in range(N_CORES):
        y[c // 4] += results[c]["yT"].T
    y += np.asarray(bo, dtype=np.float32)[None, None, :]
    return y


def kernel(**inputs):
    results = run_cores(inputs)
    return assemble(results, inputs["bo"])


# revision 10
# speedup vs baseline: 1.2561x; 1.2561x over previous
"""Trainium2 Bass kernel for causal multi-head attention (AbstractNaiveMHA).

Problem shapes (hardcoded per the harness contract):
  x: [B=2, S=2048, D=1024] f32, mask: [B, S] int32 (all ones)
  Wq/Wk/Wv: [H=16, 64, 1024], bq/bk/bv: [16, 64], Wo: [1024, 1024], bo: [1024]

Sharding over 8 NeuronCores: core c -> batch b = c // 4, head group
g = c % 4 (heads 4g..4g+3).  Each core computes its 4 heads' attention
and a partial output projection through its column slice of Wo; the
host sums the 4 partials per batch and adds bo (the "all-reduce").

v2 design notes (all matmuls bf16, rel tol 2e-2 permits it):
  - t-chunked pipeline: for each 512-token i-chunk t: project q/k/v for
    chunk t, run attention for chunk t against j-tiles 0..4t+3, then the
    output projection for chunk t.  This overlaps the ACT-engine exp
    (the phase-2 bottleneck) with projection/output matmuls and spreads
    the yT DMA across the run.
  - Scores are computed transposed (S^T[j,i]) with the two heads of a
    pair in PE row-quadrants 0:63/64:127; quadrant matmuls dual-issue
    (~95ns instead of 213ns for f=512), so k=64 costs no extra.
  - V path: vT computed v-major (weights stationary), then
    dma_start_transpose (idle DMA engines) produces the token-major
    [V_even |ones| V_odd] tiles that the AV matmul needs.
  - Causal handling: diagonal j-tiles restrict scores+exp to i >= 128r
    (saves PE stream and ACT elems); a single [zeros|tri] mask multiply
    zeroes the stale region of the es tile (ring bufs pre-zeroed once so
    stale data is always finite).
  - Softmax denominator rides free in the AV matmul via ones columns:
    av[0]=[out_e|den_e], av[1]=[den_o|out_o].  Normalization uses
    cross-partition reciprocal_approx_fast (InstReciprocal is ~7ns/elem,
    5x slower) plus two aligned muls - no flip copies.
"""

import os

import numpy as np
import ml_dtypes

import concourse.bass as bass
import concourse.mybir as mybir
import concourse.tile as tile
from concourse import bacc
from concourse.bass import ts, ds
from concourse.bass_utils import run_bass_kernel_spmd

B, S, D = 2, 2048, 1024
H, KD = 16, 64
P = 128
NT = S // 512      # 4 i-chunks of 512
NJ = S // P        # 16 j-tiles of 128
DCH = D // P       # 8 contraction chunks for the projections
N_CORES = 8
HEADS_PER_CORE = 4

F32 = mybir.dt.float32
BF16 = mybir.dt.bfloat16
EXP_SCALE = 1.0 / float(np.sqrt(np.float32(KD)))
KABL = set(os.environ.get("KABL", "").split(","))


def build_module(repeat: int = 1):
    """Build the single-core Bass module (same program on all 8 cores)."""
    nc = bacc.Bacc("TRN2", target_bir_lowering=False, debug=False)

    xT = nc.dram_tensor("xT", [D, S], BF16, kind="ExternalInput").ap()
    wq = nc.dram_tensor("wq", [2, D, P], BF16, kind="ExternalInput").ap()
    wk = nc.dram_tensor("wk", [2, D, P], BF16, kind="ExternalInput").ap()
    wv = nc.dram_tensor("wv", [D, 2 * P], BF16, kind="ExternalInput").ap()
    bqk = nc.dram_tensor("bqk", [2, 2, P], F32, kind="ExternalInput").ap()
    bv = nc.dram_tensor("bv", [P, 2], F32, kind="ExternalInput").ap()
    wo = nc.dram_tensor("wo", [P, 2, D], BF16, kind="ExternalInput").ap()
    zmask = nc.dram_tensor("zmask", [P, 1, 512], BF16,
                           kind="ExternalInput").ap()
    yT = nc.dram_tensor("yT", [D, S], BF16, kind="ExternalOutput").ap()

    Exp = mybir.ActivationFunctionType.Exp

    with tile.TileContext(nc) as tc:
        with (
            tc.tile_pool(name="cache", bufs=1) as cache,
            tc.tile_pool(name="e_pool", bufs=4) as e_pool,
            tc.tile_pool(name="c_pool", bufs=4) as c_pool,
            tc.tile_pool(name="r_pool", bufs=2) as r_pool,
            tc.tile_pool(name="y_pool", bufs=2) as y_pool,
            tc.tile_pool(name="sp_ps", bufs=2, space="PSUM") as sp_ps,
            tc.tile_pool(name="av_ps", bufs=2, space="PSUM") as av_ps,
            tc.tile_pool(name="pjq_ps", bufs=1, space="PSUM") as pjq_ps,
            tc.tile_pool(name="pjy_ps", bufs=1, space="PSUM") as pjy_ps,
        ):
            def body():
                # ---- persistent caches ----
                xT_sb = cache.tile([P, DCH, S], BF16, tag="xT")
                nc.sync.dma_start(xT_sb[:], xT.rearrange("(c p) s -> p c s", p=P))
                wq_sb = cache.tile([P, 2, DCH, P], BF16, tag="wq")
                nc.sync.dma_start(wq_sb[:], wq.rearrange("r (c p) m -> p r c m", p=P))
                wk_sb = cache.tile([P, 2, DCH, P], BF16, tag="wk")
                nc.sync.dma_start(wk_sb[:], wk.rearrange("r (c p) m -> p r c m", p=P))
                wv_sb = cache.tile([P, DCH, 2 * P], BF16, tag="wv")
                nc.sync.dma_start(wv_sb[:], wv.rearrange("(c p) n -> p c n", p=P))
                bqk_sb = cache.tile([P, 2, 2], F32, tag="bqk")
                nc.sync.dma_start(bqk_sb[:], bqk.rearrange("q r p -> p q r"))
                bv_sb = cache.tile([P, 2], F32, tag="bv")
                nc.sync.dma_start(bv_sb[:], bv[:])
                wo_sb = cache.tile([P, 2, D], BF16, tag="wo")
                nc.sync.dma_start(wo_sb[:], wo[:])
                z_sb = cache.tile([P, 1, 512], BF16, tag="z")
                nc.sync.dma_start(z_sb[:], zmask[:])

                qT_sb = cache.tile([P, 2, S], BF16, tag="qT")
                kT_sb = cache.tile([P, 2, S], BF16, tag="kT")
                vT_sb = cache.tile([P, 2, S], BF16, tag="vT")
                # per (j-tile, pair): [V_even | ones | V_odd] x 64 cols
                vones = cache.tile([P, NJ, 2, 192], BF16, tag="vones")
                nc.vector.memset(vones[:, :, :, 64:128], 1.0)

                # pre-zero the es ring so stale regions are always finite
                for i in range(4):
                    e0 = e_pool.tile([P, 2, 512], BF16, tag="e",
                                     name=f"einit{i}")
                    nc.vector.memset(e0[:], 0.0)

                # ---- work-unit emitters ----
                def qk_group(t, pr, w_sb, dstT, bcol):
                    ps = pjq_ps.tile([P, 512], F32, tag="pj",
                                     name=f"qk{t}{pr}{bcol}")
                    for c in range(DCH):
                        nc.tensor.matmul(
                            ps[:], w_sb[:, pr, c, :],
                            xT_sb[:, c, ts(t, 512)],
                            start=(c == 0), stop=(c == DCH - 1))
                    nc.vector.tensor_scalar_add(
                        dstT[:, pr, ts(t, 512)], ps[:],
                        bqk_sb[:, pr, bcol:bcol + 1])

                def v_group(t, vt):
                    ps = pjq_ps.tile([P, 512], F32, tag="pj",
                                     name=f"v{t}{vt}")
                    for c in range(DCH):
                        nc.tensor.matmul(
                            ps[:], wv_sb[:, c, ds(P * vt, P)],
                            xT_sb[:, c, ts(t, 512)],
                            start=(c == 0), stop=(c == DCH - 1))
                    nc.vector.tensor_scalar_add(
                        vT_sb[:, vt, ts(t, 512)], ps[:],
                        bv_sb[:, vt:vt + 1])
                    # transposes for this vtile's head pair (pr == vt) only:
                    # the other vtile's chunk-t data isn't drained yet
                    for r in range(4):
                        jt = 4 * t + r
                        nc.sync.dma_start_transpose(
                            vones[:, jt, vt, 0:64],
                            vT_sb[0:64, vt, ts(jt, P)])
                        nc.sync.dma_start_transpose(
                            vones[:, jt, vt, 128:192],
                            vT_sb[64:P, vt, ts(jt, P)])

                def proj_fillers(t):
                    fs = []
                    for pr in range(2):
                        for (w_sb, dstT, bcol) in ((wq_sb, qT_sb, 0),
                                                   (wk_sb, kT_sb, 1)):
                            fs.append(lambda t=t, pr=pr, w_sb=w_sb,
                                      dstT=dstT, bcol=bcol:
                                      qk_group(t, pr, w_sb, dstT, bcol))
                    for vt in range(2):
                        fs.append(lambda t=t, vt=vt: v_group(t, vt))
                    return fs

                def y_group(t, dt_, cc, y_big):
                    yps = pjy_ps.tile([P, 512], F32, tag="py",
                                      name=f"y{t}{dt_}")
                    nc.tensor.matmul(yps[:], wo_sb[:, 0, ts(dt_, P)],
                                     cc[0][:], start=True, stop=False)
                    nc.tensor.matmul(yps[:], wo_sb[:, 1, ts(dt_, P)],
                                     cc[1][:], start=False, stop=True)
                    if "noyT" not in KABL:
                        nc.vector.tensor_copy(y_big[:, dt_, :], yps[:])
                        if dt_ == DCH - 1:
                            nc.sync.dma_start(
                                yT[:, ts(t, 512)].rearrange(
                                    "(c p) s -> p c s", p=P), y_big[:])

                def phase3_fillers(t, cc):
                    y_big = y_pool.tile([P, DCH, 512], BF16, tag="y",
                                        name=f"ybig{t}")
                    return [lambda t=t, dt_=dt_, cc=cc, y_big=y_big:
                            y_group(t, dt_, cc, y_big)
                            for dt_ in range(DCH)]

                def attn_visit(t, pr, jt, njt, av):
                    r = jt - 4 * t
                    f0 = P * r if r > 0 else 0
                    sp = sp_ps.tile([P, 2, 512], F32, tag="sp",
                                    name=f"sp{t}{pr}{jt}")
                    for hi in range(2):
                        nc.tensor.matmul(
                            sp[:, hi, f0:512],
                            kT_sb[ds(64 * hi, 64), pr, ts(jt, P)],
                            qT_sb[ds(64 * hi, 64), pr,
                                  ds(512 * t + f0, 512 - f0)],
                            start=True, stop=True)
                    es = e_pool.tile([P, 2, 512], BF16, tag="e",
                                     name=f"es{t}{pr}{jt}")
                    if "noexp" not in KABL:
                        nc.scalar.activation(
                            es[:, :, f0:512], sp[:, :, f0:512],
                            Exp, scale=EXP_SCALE)
                    if r >= 0 and "nomask" not in KABL \
                            and "noexp" not in KABL:
                        w_ = P * (r + 1)
                        nc.vector.tensor_mul(
                            es[:, :, 0:w_], es[:, :, 0:w_],
                            z_sb[:, :, 512 - w_:512].to_broadcast(
                                (P, 2, w_)))
                    for hi in range(2):
                        # lhsT [V_even | ones] or [ones | V_odd]
                        nc.tensor.matmul(
                            av[hi][:],
                            vones[:, jt, pr, ds(64 * hi, P)],
                            es[:, hi, :],
                            start=(jt == 0), stop=(jt == njt - 1))

                def normalize(t, pr, av):
                    # av[0]=[out_e|den_e], av[1]=[den_o|out_o].
                    # cross-partition moves must be tensor_copy (DVE
                    # compute ops are lane-locked); 2 crossings is the
                    # minimum since out_h/den_h sit on opposite halves.
                    if "nonorm" in KABL:
                        concat = c_pool.tile([P, 512], BF16, tag="cc",
                                             name=f"cc{t}{pr}")
                        nc.vector.tensor_copy(concat[0:64, :],
                                              av[0][0:64, :])
                        nc.vector.tensor_copy(concat[64:P, :],
                                              av[1][64:P, :])
                        return concat
                    w = r_pool.tile([P, 512], F32, tag="w", name=f"w{t}{pr}")
                    nc.vector.tensor_copy(w[0:64, :], av[0][64:P, :])
                    nc.vector.tensor_copy(w[64:P, :], av[1][0:64, :])
                    rc = r_pool.tile([P, 512], F32, tag="rc",
                                     name=f"rc{t}{pr}")
                    nc.vector.reciprocal_approx_fast(rc[:], w[:])
                    concat = c_pool.tile([P, 512], BF16, tag="cc",
                                         name=f"cc{t}{pr}")
                    nc.vector.tensor_mul(
                        concat[0:64, :], av[0][0:64, :], rc[0:64, :])
                    nc.vector.tensor_mul(
                        concat[64:P, :], av[1][64:P, :], rc[64:P, :])
                    return concat

                # chunk-0 projections up front, then per chunk t: attention
                # visits with proj(t+1) and output-proj(t-1) groups
                # interleaved so the PE has filler work while ACT runs exp.
                for f in proj_fillers(0):
                    f()
                concats = {}
                for t in range(NT):
                    fillers = []
                    if t + 1 < NT:
                        fillers += proj_fillers(t + 1)
                    if t - 1 >= 0:
                        fillers += phase3_fillers(t - 1, concats[t - 1])
                    njt = 4 * t + 4
                    visits = [(pr, jt) for pr in range(2)
                              for jt in range(njt)]
                    nf = len(fillers)
                    nv = len(visits)
                    emitted = 0
                    av = None
                    for vi, (pr, jt) in enumerate(visits):
                        if jt == 0:
                            av = [av_ps.tile([P, 512], F32, tag="av",
                                             name=f"av{t}{pr}{hi}")
                                  for hi in range(2)]
                        attn_visit(t, pr, jt, njt, av)
                        if jt == njt - 1:
                            concats.setdefault(t, {})[pr] = \
                                normalize(t, pr, av)
                        while emitted * nv < (vi + 1) * nf:
                            fillers[emitted]()
                            emitted += 1
                for f in phase3_fillers(NT - 1, concats[NT - 1]):
                    f()

            if repeat > 1:
                with tc.For_i(0, repeat, 1):
                    body()
            else:
                body()

    nc.compile()
    return nc


def make_in_maps(inputs):
    bf16 = ml_dtypes.bfloat16
    x = np.asarray(inputs["x"], dtype=np.float32)
    Wq = np.asarray(inputs["Wq"], dtype=np.float32)
    bq = np.asarray(inputs["bq"], dtype=np.float32)
    Wk = np.asarray(inputs["Wk"], dtype=np.float32)
    bk = np.asarray(inputs["bk"], dtype=np.float32)
    Wv = np.asarray(inputs["Wv"], dtype=np.float32)
    bv = np.asarray(inputs["bv"], dtype=np.float32)
    Wo = np.asarray(inputs["Wo"], dtype=np.float32)

    # [zeros(384) | tri(128)]: col slice [512-128(r+1):] gives r zero
    # blocks followed by the triangular mask for the diagonal j-tile
    jj = np.arange(P)[:, None]
    ii = np.arange(P)[None, :]
    z = np.zeros((P, 1, 512), dtype=np.float32)
    z[:, 0, 384:512] = (jj <= ii).astype(np.float32)
    z = z.astype(bf16)

    in_maps = []
    for c in range(N_CORES):
        b = c // 4
        g = c % 4
        heads = list(range(HEADS_PER_CORE * g, HEADS_PER_CORE * (g + 1)))
        xT = np.ascontiguousarray(x[b].T).astype(bf16)           # [D, S]
        wq_c = np.stack([
            np.ascontiguousarray(Wq[heads[2 * p:2 * p + 2]].reshape(P, D).T)
            for p in range(2)]).astype(bf16)                      # [2, D, 128]
        wk_c = np.stack([
            np.ascontiguousarray(Wk[heads[2 * p:2 * p + 2]].reshape(P, D).T)
            for p in range(2)]).astype(bf16)
        wv_c = np.ascontiguousarray(
            Wv[heads].reshape(2 * P, D).T).astype(bf16)           # [D, 256]
        bqk = np.stack([
            bq[heads].reshape(2, P),
            bk[heads].reshape(2, P)])                             # [qk, pr, P]
        bv_c = np.stack([bv[heads[0:2]].reshape(P),
                         bv[heads[2:4]].reshape(P)], axis=1)      # [P, 2]
        # wo[c, p, d] = Wo[d, 256 g + 128 p + c]
        wo_g = Wo[:, 2 * P * g:2 * P * (g + 1)]                   # [D, 256]
        wo_c = np.ascontiguousarray(
            wo_g.T.reshape(2, P, D).transpose(1, 0, 2)).astype(bf16)
        in_maps.append({
            "xT": xT, "wq": wq_c, "wk": wk_c, "wv": wv_c,
            "bqk": np.ascontiguousarray(bqk.transpose(1, 0, 2)),  # [pr, qk, P]
            "bv": np.ascontiguousarray(bv_c), "wo": wo_c, "zmask": z,
        })
    return in_maps


_cached = {}


def _get_module(repeat: int = 1):
    if repeat not in _cached:
        _cached[repeat] = build_module(repeat)
    return _cached[repeat]


def run_cores(inputs, repeat: int = 1):
    nc = _get_module(repeat)
    in_maps = make_in_maps(inputs)
    res = run_bass_kernel_spmd(nc, in_maps, core_ids=list(range(N_CORES)))
    return res.results


def assemble(results, bo):
    y = np.zeros((B, S, D), dtype=np.float32)
    for c in range(N_CORES):
        y[c // 4] += np.asarray(results[c]["yT"], dtype=np.float32).T
    y += np.asarray(bo, dtype=np.float32)[None, None, :]
    return y


def kernel(**inputs):
    results = run_cores(inputs)
    return assemble(results, inputs["bo"])
